# revision 1
# baseline (speedup 1.0000x reference)
"""GNN message passing (HJRLConv) on 8 Trainium2 NeuronCores.

out = relu(segment_sum(edge_vals * (X @ W)[edge_src], edge_dst))
    = relu((segment_sum(edge_vals * X[edge_src], edge_dst)) @ W)

Sharding: destination nodes row-partitioned across 8 cores (12500 rows each);
edges bucketed by destination partition on the host; X replicated in bf16
(each core gathers source rows from its own full copy in local HBM, so no
halo-exchange collective is needed).

Per core:
  - edges grouped by 128-row destination block and 32768-row source range
    (dma_gather indices are int16), padded to chunks of 128 edges
  - SWDGE dma_gather fetches X_bf16[src] for ~30 chunks per instruction
    -> SBUF [128 edges, nch, 128 feat]
  - an indicator matrix S_T[e, d] = val[e] * (dst_rel[e] == d) is built on DVE
    with one fused tensor_scalar (is_equal then mult) per chunk
  - PE matmul Xg.T @ S_T accumulates aggT[feat, dst] for the dst block in PSUM
  - final fp32 matmul aggT.T @ W, ReLU on ACT, DMA to DRAM

Blocks are processed in super-blocks of 6 so each (super-block, src-range)
pair is one large gather; the 6 in-flight block accumulators plus 2 output
tiles exactly fill the 8 PSUM banks.

The chunk schedule is derived from the actual edge data and baked into the
compiled program; it is shared by all 8 cores (max over cores per
(block, range)), with val=0 padding edges keeping the program SPMD-uniform.
"""

import functools

import numpy as np
import ml_dtypes

import concourse.bacc as bacc
import concourse.bass as bass
import concourse.tile as tile
from concourse import library_config, mybir
from concourse.bass_utils import run_bass_kernel_spmd

N_NODES = 100000
N_EDGES = 1600000
D = 128
N_CORES = 8
ROWS_PER_CORE = N_NODES // N_CORES  # 12500
N_BLOCKS = (ROWS_PER_CORE + 127) // 128  # 98
PAD_ROWS = N_BLOCKS * 128  # 12544
RANGE = 32768  # dma_gather int16 index limit
N_RANGES = (N_NODES + RANGE - 1) // RANGE  # 4
SUPER = 6  # blocks per super-block (6 agg PSUM banks + 2 out banks = 8)


def _chunk_layout(cpbr):
    """Linear chunk order: super-blocks of SUPER blocks; within one,
    range-major then block-major. Returns (tot, chunk_off[b, r])."""
    chunk_off = np.zeros((N_BLOCKS, N_RANGES), dtype=np.int64)
    pos = 0
    for s0 in range(0, N_BLOCKS, SUPER):
        blocks = range(s0, min(s0 + SUPER, N_BLOCKS))
        for r in range(N_RANGES):
            for b in blocks:
                chunk_off[b, r] = pos
                pos += cpbr[b, r]
    return int(pos), chunk_off


def _schedule(edge_src, edge_vals, edge_dst):
    core = edge_dst // ROWS_PER_CORE
    counts = np.zeros((N_CORES, N_BLOCKS * N_RANGES), dtype=np.int64)
    per_core = []
    for c in range(N_CORES):
        sel = np.nonzero(core == c)[0]
        dst_l = edge_dst[sel] - c * ROWS_PER_CORE
        key = (dst_l >> 7) * N_RANGES + (edge_src[sel] >> 15)
        order = np.argsort(key, kind="stable")
        sel = sel[order]
        key = key[order]
        counts[c] = np.bincount(key, minlength=N_BLOCKS * N_RANGES)
        per_core.append((sel, key, (dst_l[order] & 127)))

    cpbr = -(-counts.max(axis=0).reshape(N_BLOCKS, N_RANGES) // 128)  # ceil
    empty = cpbr.sum(axis=1) == 0
    cpbr[empty, 0] = 1  # every block needs >=1 chunk to produce output
    tot, chunk_off = _chunk_layout(cpbr)

    idx16 = np.zeros((N_CORES, 128, tot * 8), dtype=np.int16)
    dst_T = np.zeros((N_CORES, 128, tot), dtype=np.float32)
    val_T = np.zeros((N_CORES, 128, tot), dtype=np.float32)
    slot_start = chunk_off.reshape(-1) * 128  # by key
    for c in range(N_CORES):
        sel, key, dst_rel = per_core[c]
        cnt = counts[c]
        key_start_sorted = np.concatenate([[0], np.cumsum(cnt)[:-1]])
        rank = np.arange(len(sel)) - key_start_sorted[key]
        pos = slot_start[key] + rank
        idx_flat = np.zeros(tot * 128, dtype=np.int16)
        dst_flat = np.zeros(tot * 128, dtype=np.float32)
        val_flat = np.zeros(tot * 128, dtype=np.float32)
        idx_flat[pos] = (edge_src[sel] & (RANGE - 1)).astype(np.int16)
        dst_flat[pos] = dst_rel
        val_flat[pos] = edge_vals[sel]
        # dma_gather wrapped index layout: index i -> [i % 16, i // 16],
        # replicated across the 8 groups of 16 partitions
        wrapped = idx_flat.reshape(tot * 8, 16).T  # [16, tot*8]
        idx16[c] = np.tile(wrapped, (8, 1))
        dst_T[c] = dst_flat.reshape(tot, 128).T
        val_T[c] = val_flat.reshape(tot, 128).T
    return cpbr, tot, idx16, dst_T, val_T


@functools.lru_cache(maxsize=4)
def _build_program(cpbr_key, repeat=1):
    cpbr = np.asarray(cpbr_key, dtype=np.int64).reshape(N_BLOCKS, N_RANGES)
    tot, chunk_off = _chunk_layout(cpbr)
    nch_block = cpbr.sum(axis=1)

    nc = bacc.Bacc("TRN2", target_bir_lowering=False, debug=False,
                   num_devices=N_CORES, num_swdge_queues=4)
    bf16 = mybir.dt.bfloat16
    f32 = mybir.dt.float32

    x_t = nc.dram_tensor("xbf", [N_NODES, D], bf16, kind="ExternalInput")
    w_t = nc.dram_tensor("w", [D, D], f32, kind="ExternalInput")
    iota_t = nc.dram_tensor("iota", [128, 128], f32, kind="ExternalInput")
    idx_t = nc.dram_tensor("idx", [128, tot * 8], mybir.dt.int16,
                           kind="ExternalInput")
    dst_t = nc.dram_tensor("dstrel", [128, tot], f32, kind="ExternalInput")
    val_t = nc.dram_tensor("val", [128, tot], f32, kind="ExternalInput")
    out_t = nc.dram_tensor("out", [PAD_ROWS, D], f32, kind="ExternalOutput")

    max_nch = 1
    for s0 in range(0, N_BLOCKS, SUPER):
        blocks = range(s0, min(s0 + SUPER, N_BLOCKS))
        for r in range(N_RANGES):
            max_nch = max(max_nch, int(sum(cpbr[b, r] for b in blocks)))

    with tile.TileContext(nc) as tc:
        with (
            tc.tile_pool(name="const", bufs=1) as cpool,
            tc.tile_pool(name="meta", bufs=1) as mpool,
            tc.tile_pool(name="xg", bufs=3) as xgpool,
            tc.tile_pool(name="sv", bufs=4) as svpool,
            tc.tile_pool(name="agg", bufs=3) as aggpool,
            tc.tile_pool(name="osb", bufs=3) as opool,
            tc.tile_pool(name="psA", bufs=SUPER, space="PSUM") as psa,
            tc.tile_pool(name="psB", bufs=2, space="PSUM") as psb,
        ):
            nc.gpsimd.load_library(library_config.mlp)
            w_sb = cpool.tile([128, 128], f32, tag="w")
            nc.sync.dma_start(out=w_sb[:], in_=w_t.ap())
            iota_sb = cpool.tile([128, 128], f32, tag="iota")
            nc.sync.dma_start(out=iota_sb[:], in_=iota_t.ap())
            idx_sb = mpool.tile([128, tot * 8], mybir.dt.int16, tag="idx")
            nc.sync.dma_start(out=idx_sb[:], in_=idx_t.ap())
            dst_sb = mpool.tile([128, tot], f32, tag="dst")
            nc.sync.dma_start(out=dst_sb[:], in_=dst_t.ap())
            val_sb = mpool.tile([128, tot], f32, tag="val")
            nc.sync.dma_start(out=val_sb[:], in_=val_t.ap())

            for _rep in range(repeat):
              for s0 in range(0, N_BLOCKS, SUPER):
                blocks = list(range(s0, min(s0 + SUPER, N_BLOCKS)))
                # one gather per source range covering all blocks of this
                # super-block (their chunks are contiguous in the layout)
                gathers = {}  # r -> (xg_tile, first_chunk)
                for r in range(N_RANGES):
                    nch = int(sum(cpbr[b, r] for b in blocks))
                    if nch == 0:
                        continue
                    first = int(chunk_off[blocks[0], r])
                    xg = xgpool.tile([128, max_nch, 128], bf16, tag="xg")
                    base = r * RANGE
                    rows = min(RANGE, N_NODES - base)
                    nc.gpsimd.dma_gather(
                        out_ap=xg[:, :nch, :],
                        in_ap=x_t.ap()[base : base + rows, :],
                        idxs_ap=idx_sb[:, first * 8 : (first + nch) * 8],
                        num_idxs=nch * 128,
                        num_idxs_reg=nch * 128,
                        elem_size=D,
                        single_packet=False,
                        queue_num=(s0 // SUPER * N_RANGES + r) % 4,
                    )
                    gathers[r] = (xg, first)

                psum = {b: psa.tile([128, 128], f32, tag="aggps",
                                    name=f"aggps{b}")
                        for b in blocks}
                done = {b: 0 for b in blocks}
                for r in range(N_RANGES):
                    if r not in gathers:
                        continue
                    xg, first = gathers[r]
                    for b in blocks:
                        for k in range(int(cpbr[b, r])):
                            j = int(chunk_off[b, r]) + k
                            col = j - first
                            sv = svpool.tile([128, 128], bf16, tag="sv")
                            nc.vector.tensor_scalar(
                                out=sv[:],
                                in0=iota_sb[:],
                                scalar1=dst_sb[:, j : j + 1],
                                scalar2=val_sb[:, j : j + 1],
                                op0=mybir.AluOpType.is_equal,
                                op1=mybir.AluOpType.mult,
                            )
                            nc.tensor.matmul(
                                out=psum[b][:],
                                lhsT=xg[:, col, :],
                                rhs=sv[:],
                                start=(done[b] == 0),
                                stop=(done[b] == int(nch_block[b]) - 1),
                            )
                            done[b] += 1
                            if done[b] == int(nch_block[b]):
                                agg_sb = aggpool.tile([128, 128], f32,
                                                      tag="aggsb")
                                nc.scalar.activation(
                                    out=agg_sb[:], in_=psum[b][:],
                                    func=mybir.ActivationFunctionType.Copy,
                                )
                                out_ps = psb.tile([128, 128], f32, tag="outps")
                                nc.tensor.matmul(
                                    out=out_ps[:], lhsT=agg_sb[:], rhs=w_sb[:],
                                    start=True, stop=True,
                                )
                                out_sb = opool.tile([128, 128], f32, tag="osb")
                                nc.scalar.activation(
                                    out=out_sb[:], in_=out_ps[:],
                                    func=mybir.ActivationFunctionType.Relu,
                                )
                                nc.sync.dma_start(
                                    out=out_t.ap()[b * 128 : (b + 1) * 128, :],
                                    in_=out_sb[:],
                                )

    nc.compile()
    return nc


def _prep_inputs(input_features, weight, edge_vals, edge_src, edge_dst):
    cpbr, tot, idx16, dst_T, val_T = _schedule(
        np.asarray(edge_src), np.asarray(edge_vals), np.asarray(edge_dst)
    )
    x_bf = np.asarray(input_features).astype(ml_dtypes.bfloat16)
    w = np.ascontiguousarray(np.asarray(weight, dtype=np.float32))
    iota = np.tile(np.arange(128, dtype=np.float32), (128, 1))
    in_maps = []
    for c in range(N_CORES):
        in_maps.append({
            "xbf": x_bf,
            "w": w,
            "iota": iota,
            "idx": np.ascontiguousarray(idx16[c]),
            "dstrel": np.ascontiguousarray(dst_T[c]),
            "val": np.ascontiguousarray(val_T[c]),
        })
    return cpbr, tot, in_maps


def kernel(input_features, weight, edge_vals, edge_src, edge_dst):
    cpbr, tot, in_maps = _prep_inputs(
        input_features, weight, edge_vals, edge_src, edge_dst
    )
    nc = _build_program(tuple(int(x) for x in cpbr.reshape(-1)))
    res = run_bass_kernel_spmd(nc, in_maps, list(range(N_CORES)))
    out = np.concatenate(
        [res.results[c]["out"][:ROWS_PER_CORE] for c in range(N_CORES)], axis=0
    )
    return out.astype(np.float32)



# revision 33
# speedup vs baseline: 72.7852x; 72.7852x over previous
"""GNN message passing (HJRLConv) on 8 Trainium2 NeuronCores.

out = relu(segment_sum(edge_vals * (X @ W)[edge_src], edge_dst))
    = relu((segment_sum(edge_vals * X[edge_src], edge_dst)) @ W)

Sharding: destination nodes row-partitioned across 8 cores (12500 rows each);
edges bucketed by destination partition on the host; X replicated in bf16
(each core gathers source rows from its own full copy in local HBM, so no
halo-exchange collective is needed).

Per core:
  - edges grouped by 128-row destination block and 32768-row source range
    (dma_gather indices are int16), padded to chunks of 128 edges
  - SWDGE dma_gather fetches X_bf16[src] for ~30 chunks per instruction
    -> SBUF [128 edges, nch, 128 feat]
  - the indicator matrix sv[e, d] = val[e] * (dst_rel[e] == d) is PREBUILT
    ON HOST (bf16) and streamed in with one bulk dma_start per
    (super-block, range) window, mirroring the gathers
  - PE matmul Xg.T @ sv accumulates aggT[feat, dst] for the dst block in PSUM
  - final fp32 matmul aggT.T @ W, ReLU on ACT, DMA to DRAM

Blocks are processed in super-blocks of 6 so each (super-block, src-range)
pair is one large gather; the 6 in-flight block accumulators plus 2 output
tiles exactly fill the 8 PSUM banks.

The chunk schedule is derived from the actual edge data and baked into the
compiled program; it is shared by all 8 cores (max over cores per
(block, range)), with val=0 padding edges keeping the program SPMD-uniform.
"""

import functools

import numpy as np
import ml_dtypes

import concourse.bacc as bacc
import concourse.bass as bass
import concourse.tile as tile
from concourse import library_config, mybir
from concourse.bass_utils import run_bass_kernel_spmd

N_NODES = 100000
N_EDGES = 1600000
D = 128
N_CORES = 8
ROWS_PER_CORE = N_NODES // N_CORES  # 12500
N_BLOCKS = (ROWS_PER_CORE + 127) // 128  # 98
PAD_ROWS = N_BLOCKS * 128  # 12544
RANGE = 32768  # dma_gather int16 index limit
N_RANGES = (N_NODES + RANGE - 1) // RANGE  # 4
SUPER = 6  # blocks per super-block (6 agg PSUM banks + 2 out banks = 8)


def _chunk_layout(cpbr):
    """Linear chunk order: super-blocks of SUPER blocks; within one,
    range-major then block-major. Returns (tot, chunk_off[b, r])."""
    chunk_off = np.zeros((N_BLOCKS, N_RANGES), dtype=np.int64)
    pos = 0
    for s0 in range(0, N_BLOCKS, SUPER):
        blocks = range(s0, min(s0 + SUPER, N_BLOCKS))
        for r in range(N_RANGES):
            for b in blocks:
                chunk_off[b, r] = pos
                pos += cpbr[b, r]
    return int(pos), chunk_off


def _schedule(edge_src, edge_vals, edge_dst):
    core = edge_dst // ROWS_PER_CORE
    counts = np.zeros((N_CORES, N_BLOCKS * N_RANGES), dtype=np.int64)
    per_core = []
    for c in range(N_CORES):
        sel = np.nonzero(core == c)[0]
        dst_l = edge_dst[sel] - c * ROWS_PER_CORE
        key = (dst_l >> 7) * N_RANGES + (edge_src[sel] >> 15)
        order = np.argsort(key, kind="stable")
        sel = sel[order]
        key = key[order]
        counts[c] = np.bincount(key, minlength=N_BLOCKS * N_RANGES)
        per_core.append((sel, key, (dst_l[order] & 127)))

    cpbr = -(-counts.max(axis=0).reshape(N_BLOCKS, N_RANGES) // 128)  # ceil
    empty = cpbr.sum(axis=1) == 0
    cpbr[empty, 0] = 1  # every block needs >=1 chunk to produce output
    tot, chunk_off = _chunk_layout(cpbr)

    idx16 = np.zeros((N_CORES, 128, tot * 8), dtype=np.int16)
    svf = np.zeros((N_CORES, 128, tot * 128), dtype=ml_dtypes.bfloat16)
    dst_T = np.zeros((N_CORES, 128, tot), dtype=np.float32)
    val_T = np.zeros((N_CORES, 128, tot), dtype=np.float32)
    slot_start = chunk_off.reshape(-1) * 128  # by key
    for c in range(N_CORES):
        sel, key, dst_rel = per_core[c]
        cnt = counts[c]
        key_start_sorted = np.concatenate([[0], np.cumsum(cnt)[:-1]])
        rank = np.arange(len(sel)) - key_start_sorted[key]
        pos = slot_start[key] + rank
        idx_flat = np.zeros(tot * 128, dtype=np.int16)
        idx_flat[pos] = (edge_src[sel] & (RANGE - 1)).astype(np.int16)
        # sv indicator: slot s -> (partition s%128, chunk s//128); the
        # matmul rhs column is dst_rel
        svf[c][pos % 128, (pos // 128) * 128 + dst_rel] = edge_vals[sel]
        dst_flat = np.zeros(tot * 128, dtype=np.float32)
        val_flat = np.zeros(tot * 128, dtype=np.float32)
        dst_flat[pos] = dst_rel
        val_flat[pos] = edge_vals[sel]
        dst_T[c] = dst_flat.reshape(tot, 128).T
        val_T[c] = val_flat.reshape(tot, 128).T
        # dma_gather wrapped index layout: index i -> [i % 16, i // 16],
        # replicated across the 8 groups of 16 partitions
        wrapped = idx_flat.reshape(tot * 8, 16).T  # [16, tot*8]
        idx16[c] = np.tile(wrapped, (8, 1))
    return cpbr, tot, idx16, svf, dst_T, val_T


@functools.lru_cache(maxsize=16)
def _build_program(cpbr_key, repeat=1, hw_loop=False, ablate=(),
                   xg_bufs=6, sv_bufs=6, queues=4, single_packet=False,
                   balance=False, split=1, scratch=16384, sv_mode="host",
                   idx_stream=False, sv_super=False, out_batch=False):
    cpbr = np.asarray(cpbr_key, dtype=np.int64).reshape(N_BLOCKS, N_RANGES)
    tot, chunk_off = _chunk_layout(cpbr)
    nch_block = cpbr.sum(axis=1)
    ab_gather = "gather" in ablate
    ab_sv = "sv" in ablate
    ab_mm = "mm" in ablate
    ab_tail = "tail" in ablate
    ab_wide = "wide" in ablate  # timing probe: 512B gather elems (2 rows)

    nc = bacc.Bacc("TRN2", target_bir_lowering=False, debug=False,
                   num_devices=N_CORES, num_swdge_queues=4,
                   dynamic_dma_scratch_size=scratch)

    # queue assignment: greedy least-loaded per gather piece (SPMD-safe:
    # cpbr is shared across cores)
    qload = [0] * queues
    qassign = {}  # (s0, r, piece) -> queue
    for s0 in range(0, N_BLOCKS, SUPER):
        blocks = range(s0, min(s0 + SUPER, N_BLOCKS))
        for r in range(N_RANGES):
            nch = int(sum(cpbr[b, r] for b in blocks))
            if nch == 0:
                continue
            pieces = min(split, nch)
            for p in range(pieces):
                n_p = nch // pieces + (1 if p < nch % pieces else 0)
                if balance:
                    q = min(range(queues), key=lambda i: qload[i])
                else:
                    q = (s0 // SUPER * N_RANGES + r) % queues
                qload[q] += n_p
                qassign[(s0, r, p)] = q
    bf16 = mybir.dt.bfloat16
    f32 = mybir.dt.float32

    x_t = nc.dram_tensor("xbf", [N_NODES, D], bf16, kind="ExternalInput")
    x2_t = None
    if ab_wide:
        x2_t = nc.dram_tensor("xbf2", [N_NODES // 2, 2 * D], bf16,
                              kind="ExternalInput")
    w_t = nc.dram_tensor("w", [D, D], f32, kind="ExternalInput")
    idx_t = nc.dram_tensor("idx", [128, tot * 8], mybir.dt.int16,
                           kind="ExternalInput")
    if sv_mode == "host":
        svf_t = nc.dram_tensor("svf", [128, tot * 128], bf16,
                               kind="ExternalInput")
    else:
        iota_t = nc.dram_tensor("iota", [128, 128], bf16,
                                kind="ExternalInput")
        dstv_t = nc.dram_tensor("dstv", [128, tot], f32,
                                kind="ExternalInput")
        valv_t = nc.dram_tensor("valv", [128, tot], f32,
                                kind="ExternalInput")
    out_t = nc.dram_tensor("out", [PAD_ROWS, D], f32, kind="ExternalOutput")

    max_nch = 1
    max_sup = 1
    for s0 in range(0, N_BLOCKS, SUPER):
        blocks = range(s0, min(s0 + SUPER, N_BLOCKS))
        sup_tot = 0
        for r in range(N_RANGES):
            nch = int(sum(cpbr[b, r] for b in blocks))
            max_nch = max(max_nch, nch)
            sup_tot += nch
        max_sup = max(max_sup, sup_tot)

    with tile.TileContext(nc) as tc:
        with (
            tc.tile_pool(name="const", bufs=1) as cpool,
            tc.tile_pool(name="meta", bufs=1) as mpool,
            tc.tile_pool(name="idxp", bufs=max(2, xg_bufs - 1)) as idxpool,
            tc.tile_pool(name="xg", bufs=xg_bufs) as xgpool,
            tc.tile_pool(name="sv", bufs=sv_bufs) as svpool,
            tc.tile_pool(name="agg", bufs=3) as aggpool,
            tc.tile_pool(name="osb", bufs=3) as opool,
            tc.tile_pool(name="psA", bufs=SUPER, space="PSUM") as psa,
            tc.tile_pool(name="psB", bufs=2, space="PSUM") as psb,
        ):
            nc.gpsimd.load_library(library_config.mlp)
            w_sb = cpool.tile([128, 128], f32, tag="w")
            nc.sync.dma_start(out=w_sb[:], in_=w_t.ap())
            idx_sb = None
            if not idx_stream:
                idx_sb = mpool.tile([128, tot * 8], mybir.dt.int16, tag="idx")
                nc.sync.dma_start(out=idx_sb[:], in_=idx_t.ap())
            if sv_mode != "host":
                iota_sb = mpool.tile([128, 128], bf16, tag="iota")
                nc.sync.dma_start(out=iota_sb[:], in_=iota_t.ap())
                dstv_sb = mpool.tile([128, tot], f32, tag="dstv")
                nc.sync.dma_start(out=dstv_sb[:], in_=dstv_t.ap())
                valv_sb = mpool.tile([128, tot], f32, tag="valv")
                nc.sync.dma_start(out=valv_sb[:], in_=valv_t.ap())

            xg_const = None
            if ab_gather:
                xg_const = cpool.tile([128, max_nch, 128], bf16, tag="xgc")
                for j in range(max_nch):
                    nc.sync.dma_start(out=xg_const[:, j, :],
                                      in_=x_t.ap()[0:128, :])
            sv_const = None
            if ab_sv:
                sv_const = cpool.tile([128, max_nch, 128], bf16, tag="svc")
                for j in range(max_nch):
                    nc.sync.dma_start(
                        out=sv_const[:, j, :],
                        in_=x_t.ap()[0:128, :])

            def body():
              for s0 in range(0, N_BLOCKS, SUPER):
                blocks = list(range(s0, min(s0 + SUPER, N_BLOCKS)))
                sup_first = int(chunk_off[blocks[0], 0])
                sup_nch = int(sum(cpbr[b, r] for b in blocks
                                  for r in range(N_RANGES)))
                sv_sup = None
                if sv_super and not ab_sv and sv_mode == "host":
                    sv_sup = svpool.tile([128, max_sup, 128], bf16, tag="svs")
                    nc.sync.dma_start(
                        out=sv_sup[:, :sup_nch, :],
                        in_=svf_t.ap()[:, sup_first * 128
                                       : (sup_first + sup_nch) * 128],
                    )
                osb_sup = None
                if out_batch and not (ab_tail or ab_mm):
                    osb_sup = opool.tile([128, SUPER, 128], f32, tag="osbs")
                # one gather + one bulk sv load per source range covering all
                # blocks of this super-block (their chunks are contiguous)
                gathers = {}  # r -> (xg_tile, sv_tile, first_chunk)
                for r in range(N_RANGES):
                    nch = int(sum(cpbr[b, r] for b in blocks))
                    if nch == 0:
                        continue
                    first = int(chunk_off[blocks[0], r])
                    if ab_gather:
                        xg = xg_const
                    else:
                        ew = 2 if ab_wide else 1  # rows per gathered elem
                        xg = xgpool.tile([128, max_nch, 128 * ew], bf16,
                                         tag="xg")
                        base = r * RANGE
                        rows = min(RANGE, N_NODES - base)
                        in_ap = (x2_t.ap() if ab_wide
                                 else x_t.ap()[base : base + rows, :])
                        if idx_stream:
                            idxw = idxpool.tile([128, max_nch * 8],
                                                mybir.dt.int16, tag="idxw")
                            nc.sync.dma_start(
                                out=idxw[:, : nch * 8],
                                in_=idx_t.ap()[:, first * 8 : (first + nch) * 8],
                            )
                        pieces = min(split, nch)
                        c0 = 0
                        for p in range(pieces):
                            n_p = nch // pieces + (1 if p < nch % pieces else 0)
                            nc.gpsimd.dma_gather(
                                out_ap=xg[:, c0 : c0 + n_p, :],
                                in_ap=in_ap,
                                idxs_ap=(
                                    idxw[:, c0 * 8 : (c0 + n_p) * 8]
                                    if idx_stream else
                                    idx_sb[
                                        :, (first + c0) * 8 : (first + c0 + n_p) * 8]),
                                num_idxs=n_p * 128,
                                num_idxs_reg=n_p * 128,
                                elem_size=D * ew,
                                single_packet=single_packet,
                                queue_num=qassign[(s0, r, p)],
                            )
                            c0 += n_p
                    if ab_sv:
                        sv = sv_const
                    elif sv_mode != "host":
                        sv = None
                    elif sv_super:
                        sv = None  # use sv_sup
                    else:
                        sv = svpool.tile([128, max_nch, 128], bf16, tag="sv")
                        nc.sync.dma_start(
                            out=sv[:, :nch, :],
                            in_=svf_t.ap()[:, first * 128 : (first + nch) * 128],
                        )
                    gathers[r] = (xg, sv, first)

                psum = {b: psa.tile([128, 128], f32, tag="aggps",
                                    name=f"aggps{b}")
                        for b in blocks}
                done = {b: 0 for b in blocks}
                for r in range(N_RANGES):
                    if r not in gathers:
                        continue
                    xg, sv, first = gathers[r]
                    for b in blocks:
                        for k in range(int(cpbr[b, r])):
                            j = int(chunk_off[b, r]) + k
                            col = j - first
                            if ab_sv:
                                rhs = sv[:, col, :]
                            elif sv_mode == "host":
                                if sv_super:
                                    rhs = sv_sup[:, j - sup_first, :]
                                else:
                                    rhs = sv[:, col, :]
                            else:
                                svc = svpool.tile([128, 128], bf16, tag="svd")
                                nc.vector.tensor_scalar(
                                    out=svc[:],
                                    in0=iota_sb[:],
                                    scalar1=dstv_sb[:, j : j + 1],
                                    scalar2=valv_sb[:, j : j + 1],
                                    op0=mybir.AluOpType.is_equal,
                                    op1=mybir.AluOpType.mult,
                                )
                                rhs = svc[:]
                            if not ab_mm:
                                nc.tensor.matmul(
                                    out=psum[b][:],
                                    lhsT=xg[:, col, :],
                                    rhs=rhs,
                                    start=(done[b] == 0),
                                    stop=(done[b] == int(nch_block[b]) - 1),
                                )
                            done[b] += 1
                            if done[b] == int(nch_block[b]) and not (
                                    ab_tail or ab_mm):
                                agg_sb = aggpool.tile([128, 128], f32,
                                                      tag="aggsb")
                                nc.scalar.activation(
                                    out=agg_sb[:], in_=psum[b][:],
                                    func=mybir.ActivationFunctionType.Copy,
                                )
                                out_ps = psb.tile([128, 128], f32, tag="outps")
                                nc.tensor.matmul(
                                    out=out_ps[:], lhsT=agg_sb[:], rhs=w_sb[:],
                                    start=True, stop=True,
                                )
                                if out_batch:
                                    nc.scalar.activation(
                                        out=osb_sup[:, b - s0, :],
                                        in_=out_ps[:],
                                        func=mybir.ActivationFunctionType.Relu,
                                    )
                                else:
                                    out_sb = opool.tile([128, 128], f32,
                                                        tag="osb")
                                    nc.scalar.activation(
                                        out=out_sb[:], in_=out_ps[:],
                                        func=mybir.ActivationFunctionType.Relu,
                                    )
                                    nc.sync.dma_start(
                                        out=out_t.ap()[
                                            b * 128 : (b + 1) * 128, :],
                                        in_=out_sb[:],
                                    )
                if out_batch and not (ab_tail or ab_mm):
                    nb = len(blocks)
                    out_view = out_t.ap()[
                        s0 * 128 : (s0 + nb) * 128, :
                    ].rearrange("(j p) f -> p j f", p=128)
                    nc.sync.dma_start(
                        out=out_view,
                        in_=osb_sup[:, :nb, :],
                    )

            if hw_loop and repeat > 1:
                with tc.For_i(0, repeat, 1):
                    body()
            else:
                for _rep in range(repeat):
                    body()

    nc.compile()
    return nc


def _prep_inputs(input_features, weight, edge_vals, edge_src, edge_dst):
    cpbr, tot, idx16, svf, dst_T, val_T = _schedule(
        np.asarray(edge_src), np.asarray(edge_vals), np.asarray(edge_dst)
    )
    x_bf = np.asarray(input_features).astype(ml_dtypes.bfloat16)
    w = np.ascontiguousarray(np.asarray(weight, dtype=np.float32))
    iota = np.tile(np.arange(128, dtype=np.float32), (128, 1)).astype(
        ml_dtypes.bfloat16)
    in_maps = []
    for c in range(N_CORES):
        in_maps.append({
            "xbf": x_bf,
            "w": w,
            "iota": iota,
            "idx": np.ascontiguousarray(idx16[c]),
            "svf": np.ascontiguousarray(svf[c]),
            "dstv": np.ascontiguousarray(dst_T[c]),
            "valv": np.ascontiguousarray(val_T[c]),
        })
    return cpbr, tot, in_maps


# best-known program configuration (updated as benchmarks come in)
BEST = dict(balance=True, split=2)


def kernel(input_features, weight, edge_vals, edge_src, edge_dst):
    cpbr, tot, in_maps = _prep_inputs(
        input_features, weight, edge_vals, edge_src, edge_dst
    )
    nc = _build_program(tuple(int(x) for x in cpbr.reshape(-1)), **BEST)
    res = run_bass_kernel_spmd(nc, in_maps, list(range(N_CORES)))
    out = np.concatenate(
        [res.results[c]["out"][:ROWS_PER_CORE] for c in range(N_CORES)], axis=0
    )
    return out.astype(np.float32)


# revision 35
# speedup vs baseline: 79.4191x; 1.0911x over previous
"""GNN message passing (HJRLConv) on 8 Trainium2 NeuronCores.

out = relu(segment_sum(edge_vals * (X @ W)[edge_src], edge_dst))
    = relu((segment_sum(edge_vals * X[edge_src], edge_dst)) @ W)

Sharding: destination nodes row-partitioned across 8 cores (12500 rows each);
edges bucketed by destination partition on the host; X replicated in bf16
(each core gathers source rows from its own full copy in local HBM, so no
halo-exchange collective is needed).

Per core:
  - edges grouped by 128-row destination block and 32768-row source range
    (dma_gather indices are int16), padded to chunks of 128 edges
  - SWDGE dma_gather fetches X_bf16[src], two instructions per
    (super-block, range) window -> SBUF [128 edges, nch, 128 feat].
    Gather pieces are greedily balanced across the 4 SWDGE queues: each
    queue's descriptors are generated by a different Q7 core pair
    (cpu_id/2 == queue_num), so balancing parallelizes descriptor
    generation — the kernel's critical path — 4-wide.
  - the indicator matrix sv[e, d] = val[e] * (dst_rel[e] == d) is PREBUILT
    ON HOST (bf16) and streamed in with one bulk dma_start per super-block
    (cheap sequential DMA instead of per-chunk DVE tensor_scalar ops)
  - PE matmul Xg.T @ sv accumulates aggT[feat, dst] for the dst block in PSUM
  - final fp32 matmul aggT.T @ W, ReLU on ACT, one batched output DMA per
    super-block to DRAM

Blocks are processed in super-blocks of 6 so each (super-block, src-range)
pair is one large gather; the 6 in-flight block accumulators plus 2 output
tiles exactly fill the 8 PSUM banks.

The chunk schedule is derived from the actual edge data and baked into the
compiled program; it is shared by all 8 cores (max over cores per
(block, range)), with val=0 padding edges keeping the program SPMD-uniform.
"""

import functools

import numpy as np
import ml_dtypes

import concourse.bacc as bacc
import concourse.bass as bass
import concourse.tile as tile
from concourse import library_config, mybir
from concourse.bass_utils import run_bass_kernel_spmd

N_NODES = 100000
N_EDGES = 1600000
D = 128
N_CORES = 8
ROWS_PER_CORE = N_NODES // N_CORES  # 12500
N_BLOCKS = (ROWS_PER_CORE + 127) // 128  # 98
PAD_ROWS = N_BLOCKS * 128  # 12544
RANGE = 32768  # dma_gather int16 index limit
N_RANGES = (N_NODES + RANGE - 1) // RANGE  # 4
SUPER = 6  # blocks per super-block (6 agg PSUM banks + 2 out banks = 8)


def _chunk_layout(cpbr):
    """Linear chunk order: super-blocks of SUPER blocks; within one,
    range-major then block-major. Returns (tot, chunk_off[b, r])."""
    chunk_off = np.zeros((N_BLOCKS, N_RANGES), dtype=np.int64)
    pos = 0
    for s0 in range(0, N_BLOCKS, SUPER):
        blocks = range(s0, min(s0 + SUPER, N_BLOCKS))
        for r in range(N_RANGES):
            for b in blocks:
                chunk_off[b, r] = pos
                pos += cpbr[b, r]
    return int(pos), chunk_off


def _schedule(edge_src, edge_vals, edge_dst):
    core = edge_dst // ROWS_PER_CORE
    counts = np.zeros((N_CORES, N_BLOCKS * N_RANGES), dtype=np.int64)
    per_core = []
    for c in range(N_CORES):
        sel = np.nonzero(core == c)[0]
        dst_l = edge_dst[sel] - c * ROWS_PER_CORE
        key = (dst_l >> 7) * N_RANGES + (edge_src[sel] >> 15)
        order = np.argsort(key, kind="stable")
        sel = sel[order]
        key = key[order]
        counts[c] = np.bincount(key, minlength=N_BLOCKS * N_RANGES)
        per_core.append((sel, key, (dst_l[order] & 127)))

    cpbr = -(-counts.max(axis=0).reshape(N_BLOCKS, N_RANGES) // 128)  # ceil
    empty = cpbr.sum(axis=1) == 0
    cpbr[empty, 0] = 1  # every block needs >=1 chunk to produce output
    tot, chunk_off = _chunk_layout(cpbr)

    idx16 = np.zeros((N_CORES, 128, tot * 8), dtype=np.int16)
    svf = np.zeros((N_CORES, 128, tot * 128), dtype=ml_dtypes.bfloat16)
    dst_T = np.zeros((N_CORES, 128, tot), dtype=np.float32)
    val_T = np.zeros((N_CORES, 128, tot), dtype=np.float32)
    slot_start = chunk_off.reshape(-1) * 128  # by key
    for c in range(N_CORES):
        sel, key, dst_rel = per_core[c]
        cnt = counts[c]
        key_start_sorted = np.concatenate([[0], np.cumsum(cnt)[:-1]])
        rank = np.arange(len(sel)) - key_start_sorted[key]
        pos = slot_start[key] + rank
        idx_flat = np.zeros(tot * 128, dtype=np.int16)
        idx_flat[pos] = (edge_src[sel] & (RANGE - 1)).astype(np.int16)
        # sv indicator: slot s -> (partition s%128, chunk s//128); the
        # matmul rhs column is dst_rel
        svf[c][pos % 128, (pos // 128) * 128 + dst_rel] = edge_vals[sel]
        dst_flat = np.zeros(tot * 128, dtype=np.float32)
        val_flat = np.zeros(tot * 128, dtype=np.float32)
        dst_flat[pos] = dst_rel
        val_flat[pos] = edge_vals[sel]
        dst_T[c] = dst_flat.reshape(tot, 128).T
        val_T[c] = val_flat.reshape(tot, 128).T
        # dma_gather wrapped index layout: index i -> [i % 16, i // 16],
        # replicated across the 8 groups of 16 partitions
        wrapped = idx_flat.reshape(tot * 8, 16).T  # [16, tot*8]
        idx16[c] = np.tile(wrapped, (8, 1))
    return cpbr, tot, idx16, svf, dst_T, val_T


@functools.lru_cache(maxsize=16)
def _build_program(cpbr_key, repeat=1, hw_loop=False, ablate=(),
                   xg_bufs=6, sv_bufs=6, queues=4, single_packet=False,
                   balance=False, split=1, scratch=16384, sv_mode="host",
                   idx_stream=False, sv_super=False, out_batch=False):
    cpbr = np.asarray(cpbr_key, dtype=np.int64).reshape(N_BLOCKS, N_RANGES)
    tot, chunk_off = _chunk_layout(cpbr)
    nch_block = cpbr.sum(axis=1)
    ab_gather = "gather" in ablate
    ab_sv = "sv" in ablate
    ab_mm = "mm" in ablate
    ab_tail = "tail" in ablate
    ab_wide = "wide" in ablate  # timing probe: 512B gather elems (2 rows)

    nc = bacc.Bacc("TRN2", target_bir_lowering=False, debug=False,
                   num_devices=N_CORES, num_swdge_queues=4,
                   dynamic_dma_scratch_size=scratch)

    # queue assignment: greedy least-loaded per gather piece (SPMD-safe:
    # cpbr is shared across cores)
    qload = [0] * queues
    qassign = {}  # (s0, r, piece) -> queue
    for s0 in range(0, N_BLOCKS, SUPER):
        blocks = range(s0, min(s0 + SUPER, N_BLOCKS))
        for r in range(N_RANGES):
            nch = int(sum(cpbr[b, r] for b in blocks))
            if nch == 0:
                continue
            pieces = min(split, nch)
            for p in range(pieces):
                n_p = nch // pieces + (1 if p < nch % pieces else 0)
                if balance:
                    q = min(range(queues), key=lambda i: qload[i])
                else:
                    q = (s0 // SUPER * N_RANGES + r) % queues
                qload[q] += n_p
                qassign[(s0, r, p)] = q
    bf16 = mybir.dt.bfloat16
    f32 = mybir.dt.float32

    x_t = nc.dram_tensor("xbf", [N_NODES, D], bf16, kind="ExternalInput")
    x2_t = None
    if ab_wide:
        x2_t = nc.dram_tensor("xbf2", [N_NODES // 2, 2 * D], bf16,
                              kind="ExternalInput")
    w_t = nc.dram_tensor("w", [D, D], f32, kind="ExternalInput")
    idx_t = nc.dram_tensor("idx", [128, tot * 8], mybir.dt.int16,
                           kind="ExternalInput")
    if sv_mode == "host":
        svf_t = nc.dram_tensor("svf", [128, tot * 128], bf16,
                               kind="ExternalInput")
    else:
        iota_t = nc.dram_tensor("iota", [128, 128], bf16,
                                kind="ExternalInput")
        dstv_t = nc.dram_tensor("dstv", [128, tot], f32,
                                kind="ExternalInput")
        valv_t = nc.dram_tensor("valv", [128, tot], f32,
                                kind="ExternalInput")
    out_t = nc.dram_tensor("out", [PAD_ROWS, D], f32, kind="ExternalOutput")

    max_nch = 1
    max_sup = 1
    for s0 in range(0, N_BLOCKS, SUPER):
        blocks = range(s0, min(s0 + SUPER, N_BLOCKS))
        sup_tot = 0
        for r in range(N_RANGES):
            nch = int(sum(cpbr[b, r] for b in blocks))
            max_nch = max(max_nch, nch)
            sup_tot += nch
        max_sup = max(max_sup, sup_tot)

    with tile.TileContext(nc) as tc:
        with (
            tc.tile_pool(name="const", bufs=1) as cpool,
            tc.tile_pool(name="meta", bufs=1) as mpool,
            tc.tile_pool(name="idxp", bufs=max(2, xg_bufs - 1)) as idxpool,
            tc.tile_pool(name="xg", bufs=xg_bufs) as xgpool,
            tc.tile_pool(name="sv", bufs=sv_bufs) as svpool,
            tc.tile_pool(name="agg", bufs=3) as aggpool,
            tc.tile_pool(name="osb", bufs=3) as opool,
            tc.tile_pool(name="psA", bufs=SUPER, space="PSUM") as psa,
            tc.tile_pool(name="psB", bufs=2, space="PSUM") as psb,
        ):
            nc.gpsimd.load_library(library_config.mlp)
            w_sb = cpool.tile([128, 128], f32, tag="w")
            nc.sync.dma_start(out=w_sb[:], in_=w_t.ap())
            idx_sb = None
            if not idx_stream:
                idx_sb = mpool.tile([128, tot * 8], mybir.dt.int16, tag="idx")
                nc.sync.dma_start(out=idx_sb[:], in_=idx_t.ap())
            if sv_mode != "host":
                iota_sb = mpool.tile([128, 128], bf16, tag="iota")
                nc.sync.dma_start(out=iota_sb[:], in_=iota_t.ap())
                dstv_sb = mpool.tile([128, tot], f32, tag="dstv")
                nc.sync.dma_start(out=dstv_sb[:], in_=dstv_t.ap())
                valv_sb = mpool.tile([128, tot], f32, tag="valv")
                nc.sync.dma_start(out=valv_sb[:], in_=valv_t.ap())

            xg_const = None
            if ab_gather:
                xg_const = cpool.tile([128, max_nch, 128], bf16, tag="xgc")
                for j in range(max_nch):
                    nc.sync.dma_start(out=xg_const[:, j, :],
                                      in_=x_t.ap()[0:128, :])
            sv_const = None
            if ab_sv:
                sv_const = cpool.tile([128, max_nch, 128], bf16, tag="svc")
                for j in range(max_nch):
                    nc.sync.dma_start(
                        out=sv_const[:, j, :],
                        in_=x_t.ap()[0:128, :])

            def body():
              for s0 in range(0, N_BLOCKS, SUPER):
                blocks = list(range(s0, min(s0 + SUPER, N_BLOCKS)))
                sup_first = int(chunk_off[blocks[0], 0])
                sup_nch = int(sum(cpbr[b, r] for b in blocks
                                  for r in range(N_RANGES)))
                sv_sup = None
                if sv_super and not ab_sv and sv_mode == "host":
                    sv_sup = svpool.tile([128, max_sup, 128], bf16, tag="svs")
                    nc.sync.dma_start(
                        out=sv_sup[:, :sup_nch, :],
                        in_=svf_t.ap()[:, sup_first * 128
                                       : (sup_first + sup_nch) * 128],
                    )
                osb_sup = None
                if out_batch and not (ab_tail or ab_mm):
                    osb_sup = opool.tile([128, SUPER, 128], f32, tag="osbs")
                # one gather + one bulk sv load per source range covering all
                # blocks of this super-block (their chunks are contiguous)
                gathers = {}  # r -> (xg_tile, sv_tile, first_chunk)
                for r in range(N_RANGES):
                    nch = int(sum(cpbr[b, r] for b in blocks))
                    if nch == 0:
                        continue
                    first = int(chunk_off[blocks[0], r])
                    if ab_gather:
                        xg = xg_const
                    else:
                        ew = 2 if ab_wide else 1  # rows per gathered elem
                        xg = xgpool.tile([128, max_nch, 128 * ew], bf16,
                                         tag="xg")
                        base = r * RANGE
                        rows = min(RANGE, N_NODES - base)
                        in_ap = (x2_t.ap() if ab_wide
                                 else x_t.ap()[base : base + rows, :])
                        if idx_stream:
                            idxw = idxpool.tile([128, max_nch * 8],
                                                mybir.dt.int16, tag="idxw")
                            nc.sync.dma_start(
                                out=idxw[:, : nch * 8],
                                in_=idx_t.ap()[:, first * 8 : (first + nch) * 8],
                            )
                        pieces = min(split, nch)
                        c0 = 0
                        for p in range(pieces):
                            n_p = nch // pieces + (1 if p < nch % pieces else 0)
                            nc.gpsimd.dma_gather(
                                out_ap=xg[:, c0 : c0 + n_p, :],
                                in_ap=in_ap,
                                idxs_ap=(
                                    idxw[:, c0 * 8 : (c0 + n_p) * 8]
                                    if idx_stream else
                                    idx_sb[
                                        :, (first + c0) * 8 : (first + c0 + n_p) * 8]),
                                num_idxs=n_p * 128,
                                num_idxs_reg=n_p * 128,
                                elem_size=D * ew,
                                single_packet=single_packet,
                                queue_num=qassign[(s0, r, p)],
                            )
                            c0 += n_p
                    if ab_sv:
                        sv = sv_const
                    elif sv_mode != "host":
                        sv = None
                    elif sv_super:
                        sv = None  # use sv_sup
                    else:
                        sv = svpool.tile([128, max_nch, 128], bf16, tag="sv")
                        nc.sync.dma_start(
                            out=sv[:, :nch, :],
                            in_=svf_t.ap()[:, first * 128 : (first + nch) * 128],
                        )
                    gathers[r] = (xg, sv, first)

                psum = {b: psa.tile([128, 128], f32, tag="aggps",
                                    name=f"aggps{b}")
                        for b in blocks}
                done = {b: 0 for b in blocks}
                for r in range(N_RANGES):
                    if r not in gathers:
                        continue
                    xg, sv, first = gathers[r]
                    for b in blocks:
                        for k in range(int(cpbr[b, r])):
                            j = int(chunk_off[b, r]) + k
                            col = j - first
                            if ab_sv:
                                rhs = sv[:, col, :]
                            elif sv_mode == "host":
                                if sv_super:
                                    rhs = sv_sup[:, j - sup_first, :]
                                else:
                                    rhs = sv[:, col, :]
                            else:
                                svc = svpool.tile([128, 128], bf16, tag="svd")
                                nc.vector.tensor_scalar(
                                    out=svc[:],
                                    in0=iota_sb[:],
                                    scalar1=dstv_sb[:, j : j + 1],
                                    scalar2=valv_sb[:, j : j + 1],
                                    op0=mybir.AluOpType.is_equal,
                                    op1=mybir.AluOpType.mult,
                                )
                                rhs = svc[:]
                            if not ab_mm:
                                nc.tensor.matmul(
                                    out=psum[b][:],
                                    lhsT=xg[:, col, :],
                                    rhs=rhs,
                                    start=(done[b] == 0),
                                    stop=(done[b] == int(nch_block[b]) - 1),
                                )
                            done[b] += 1
                            if done[b] == int(nch_block[b]) and not (
                                    ab_tail or ab_mm):
                                agg_sb = aggpool.tile([128, 128], f32,
                                                      tag="aggsb")
                                nc.scalar.activation(
                                    out=agg_sb[:], in_=psum[b][:],
                                    func=mybir.ActivationFunctionType.Copy,
                                )
                                out_ps = psb.tile([128, 128], f32, tag="outps")
                                nc.tensor.matmul(
                                    out=out_ps[:], lhsT=agg_sb[:], rhs=w_sb[:],
                                    start=True, stop=True,
                                )
                                if out_batch:
                                    nc.scalar.activation(
                                        out=osb_sup[:, b - s0, :],
                                        in_=out_ps[:],
                                        func=mybir.ActivationFunctionType.Relu,
                                    )
                                else:
                                    out_sb = opool.tile([128, 128], f32,
                                                        tag="osb")
                                    nc.scalar.activation(
                                        out=out_sb[:], in_=out_ps[:],
                                        func=mybir.ActivationFunctionType.Relu,
                                    )
                                    nc.sync.dma_start(
                                        out=out_t.ap()[
                                            b * 128 : (b + 1) * 128, :],
                                        in_=out_sb[:],
                                    )
                if out_batch and not (ab_tail or ab_mm):
                    nb = len(blocks)
                    out_view = out_t.ap()[
                        s0 * 128 : (s0 + nb) * 128, :
                    ].rearrange("(j p) f -> p j f", p=128)
                    nc.sync.dma_start(
                        out=out_view,
                        in_=osb_sup[:, :nb, :],
                    )

            if hw_loop and repeat > 1:
                with tc.For_i(0, repeat, 1):
                    body()
            else:
                for _rep in range(repeat):
                    body()

    nc.compile()
    return nc


def _prep_inputs(input_features, weight, edge_vals, edge_src, edge_dst):
    cpbr, tot, idx16, svf, dst_T, val_T = _schedule(
        np.asarray(edge_src), np.asarray(edge_vals), np.asarray(edge_dst)
    )
    x_bf = np.asarray(input_features).astype(ml_dtypes.bfloat16)
    w = np.ascontiguousarray(np.asarray(weight, dtype=np.float32))
    iota = np.tile(np.arange(128, dtype=np.float32), (128, 1)).astype(
        ml_dtypes.bfloat16)
    in_maps = []
    for c in range(N_CORES):
        in_maps.append({
            "xbf": x_bf,
            "w": w,
            "iota": iota,
            "idx": np.ascontiguousarray(idx16[c]),
            "svf": np.ascontiguousarray(svf[c]),
            "dstv": np.ascontiguousarray(dst_T[c]),
            "valv": np.ascontiguousarray(val_T[c]),
        })
    return cpbr, tot, in_maps


# best-known program configuration (updated as benchmarks come in)
BEST = dict(balance=True, split=2, sv_super=True, sv_bufs=2, out_batch=True)


def kernel(input_features, weight, edge_vals, edge_src, edge_dst):
    cpbr, tot, in_maps = _prep_inputs(
        input_features, weight, edge_vals, edge_src, edge_dst
    )
    nc = _build_program(tuple(int(x) for x in cpbr.reshape(-1)), **BEST)
    res = run_bass_kernel_spmd(nc, in_maps, list(range(N_CORES)))
    out = np.concatenate(
        [res.results[c]["out"][:ROWS_PER_CORE] for c in range(N_CORES)], axis=0
    )
    return out.astype(np.float32)


# revision 38
# speedup vs baseline: 80.2539x; 1.0105x over previous
"""GNN message passing (HJRLConv) on 8 Trainium2 NeuronCores.

out = relu(segment_sum(edge_vals * (X @ W)[edge_src], edge_dst))
    = relu((segment_sum(edge_vals * X[edge_src], edge_dst)) @ W)

Sharding: destination nodes row-partitioned across 8 cores (12500 rows each);
edges bucketed by destination partition on the host; X replicated in bf16
(each core gathers source rows from its own full copy in local HBM, so no
halo-exchange collective is needed).

Per core:
  - edges grouped by 128-row destination block and 32768-row source range
    (dma_gather indices are int16), padded to chunks of 128 edges
  - SWDGE dma_gather fetches X_bf16[src], two instructions per
    (super-block, range) window -> SBUF [128 edges, nch, 128 feat].
    Gather pieces are greedily balanced across the 4 SWDGE queues: each
    queue's descriptors are generated by a different Q7 core pair
    (cpu_id/2 == queue_num), so balancing parallelizes descriptor
    generation — the kernel's critical path — 4-wide.
  - the indicator matrix sv[e, d] = val[e] * (dst_rel[e] == d) is PREBUILT
    ON HOST (bf16) and streamed in with one bulk dma_start per super-block
    (cheap sequential DMA instead of per-chunk DVE tensor_scalar ops)
  - PE matmul Xg.T @ sv accumulates aggT[feat, dst] for the dst block in PSUM
  - final fp32 matmul aggT.T @ W, ReLU on ACT, one batched output DMA per
    super-block to DRAM

Blocks are processed in super-blocks of 6 so each (super-block, src-range)
pair is one large gather; the 6 in-flight block accumulators plus 2 output
tiles exactly fill the 8 PSUM banks.

The chunk schedule is derived from the actual edge data and baked into the
compiled program; it is shared by all 8 cores (max over cores per
(block, range)), with val=0 padding edges keeping the program SPMD-uniform.
"""

import functools

import numpy as np
import ml_dtypes

import concourse.bacc as bacc
import concourse.bass as bass
import concourse.tile as tile
from concourse import library_config, mybir
from concourse.bass_utils import run_bass_kernel_spmd

N_NODES = 100000
N_EDGES = 1600000
D = 128
N_CORES = 8
ROWS_PER_CORE = N_NODES // N_CORES  # 12500
N_BLOCKS = (ROWS_PER_CORE + 127) // 128  # 98
PAD_ROWS = N_BLOCKS * 128  # 12544
RANGE = 32768  # dma_gather int16 index limit
N_RANGES = (N_NODES + RANGE - 1) // RANGE  # 4
SUPER = 6  # blocks per super-block (6 agg PSUM banks + 2 out banks = 8)


def _chunk_layout(cpbr):
    """Linear chunk order: super-blocks of SUPER blocks; within one,
    range-major then block-major. Returns (tot, chunk_off[b, r])."""
    chunk_off = np.zeros((N_BLOCKS, N_RANGES), dtype=np.int64)
    pos = 0
    for s0 in range(0, N_BLOCKS, SUPER):
        blocks = range(s0, min(s0 + SUPER, N_BLOCKS))
        for r in range(N_RANGES):
            for b in blocks:
                chunk_off[b, r] = pos
                pos += cpbr[b, r]
    return int(pos), chunk_off


def _schedule(edge_src, edge_vals, edge_dst):
    core = edge_dst // ROWS_PER_CORE
    counts = np.zeros((N_CORES, N_BLOCKS * N_RANGES), dtype=np.int64)
    per_core = []
    for c in range(N_CORES):
        sel = np.nonzero(core == c)[0]
        dst_l = edge_dst[sel] - c * ROWS_PER_CORE
        key = (dst_l >> 7) * N_RANGES + (edge_src[sel] >> 15)
        order = np.argsort(key, kind="stable")
        sel = sel[order]
        key = key[order]
        counts[c] = np.bincount(key, minlength=N_BLOCKS * N_RANGES)
        per_core.append((sel, key, (dst_l[order] & 127)))

    cpbr = -(-counts.max(axis=0).reshape(N_BLOCKS, N_RANGES) // 128)  # ceil
    empty = cpbr.sum(axis=1) == 0
    cpbr[empty, 0] = 1  # every block needs >=1 chunk to produce output
    tot, chunk_off = _chunk_layout(cpbr)

    idx16 = np.zeros((N_CORES, 128, tot * 8), dtype=np.int16)
    svf = np.zeros((N_CORES, 128, tot * 128), dtype=ml_dtypes.bfloat16)
    dst_T = np.zeros((N_CORES, 128, tot), dtype=np.float32)
    val_T = np.zeros((N_CORES, 128, tot), dtype=np.float32)
    slot_start = chunk_off.reshape(-1) * 128  # by key
    for c in range(N_CORES):
        sel, key, dst_rel = per_core[c]
        cnt = counts[c]
        key_start_sorted = np.concatenate([[0], np.cumsum(cnt)[:-1]])
        rank = np.arange(len(sel)) - key_start_sorted[key]
        pos = slot_start[key] + rank
        idx_flat = np.zeros(tot * 128, dtype=np.int16)
        idx_flat[pos] = (edge_src[sel] & (RANGE - 1)).astype(np.int16)
        # sv indicator: slot s -> (partition s%128, chunk s//128); the
        # matmul rhs column is dst_rel
        svf[c][pos % 128, (pos // 128) * 128 + dst_rel] = edge_vals[sel]
        dst_flat = np.zeros(tot * 128, dtype=np.float32)
        val_flat = np.zeros(tot * 128, dtype=np.float32)
        dst_flat[pos] = dst_rel
        val_flat[pos] = edge_vals[sel]
        dst_T[c] = dst_flat.reshape(tot, 128).T
        val_T[c] = val_flat.reshape(tot, 128).T
        # dma_gather wrapped index layout: index i -> [i % 16, i // 16],
        # replicated across the 8 groups of 16 partitions
        wrapped = idx_flat.reshape(tot * 8, 16).T  # [16, tot*8]
        idx16[c] = np.tile(wrapped, (8, 1))
    return cpbr, tot, idx16, svf, dst_T, val_T


@functools.lru_cache(maxsize=16)
def _build_program(cpbr_key, repeat=1, hw_loop=False, ablate=(),
                   xg_bufs=6, sv_bufs=6, queues=4, single_packet=False,
                   balance=False, split=1, scratch=16384, sv_mode="host",
                   idx_stream=False, sv_super=False, out_batch=False,
                   hints=False):
    cpbr = np.asarray(cpbr_key, dtype=np.int64).reshape(N_BLOCKS, N_RANGES)
    tot, chunk_off = _chunk_layout(cpbr)
    nch_block = cpbr.sum(axis=1)
    ab_gather = "gather" in ablate
    ab_sv = "sv" in ablate
    ab_mm = "mm" in ablate
    ab_tail = "tail" in ablate
    ab_wide = "wide" in ablate  # timing probe: 512B gather elems (2 rows)

    nc = bacc.Bacc("TRN2", target_bir_lowering=False, debug=False,
                   num_devices=N_CORES, num_swdge_queues=4,
                   dynamic_dma_scratch_size=scratch)

    # queue assignment: greedy least-loaded per gather piece (SPMD-safe:
    # cpbr is shared across cores)
    qload = [0] * queues
    qassign = {}  # (s0, r, piece) -> queue
    for s0 in range(0, N_BLOCKS, SUPER):
        blocks = range(s0, min(s0 + SUPER, N_BLOCKS))
        for r in range(N_RANGES):
            nch = int(sum(cpbr[b, r] for b in blocks))
            if nch == 0:
                continue
            pieces = min(split, nch)
            for p in range(pieces):
                n_p = nch // pieces + (1 if p < nch % pieces else 0)
                if balance:
                    q = min(range(queues), key=lambda i: qload[i])
                else:
                    q = (s0 // SUPER * N_RANGES + r) % queues
                qload[q] += n_p
                qassign[(s0, r, p)] = q
    bf16 = mybir.dt.bfloat16
    f32 = mybir.dt.float32

    x_t = nc.dram_tensor("xbf", [N_NODES, D], bf16, kind="ExternalInput")
    x2_t = None
    if ab_wide:
        x2_t = nc.dram_tensor("xbf2", [N_NODES // 2, 2 * D], bf16,
                              kind="ExternalInput")
    w_t = nc.dram_tensor("w", [D, D], f32, kind="ExternalInput")
    idx_t = nc.dram_tensor("idx", [128, tot * 8], mybir.dt.int16,
                           kind="ExternalInput")
    if sv_mode == "host":
        svf_t = nc.dram_tensor("svf", [128, tot * 128], bf16,
                               kind="ExternalInput")
    else:
        iota_t = nc.dram_tensor("iota", [128, 128], bf16,
                                kind="ExternalInput")
        dstv_t = nc.dram_tensor("dstv", [128, tot], f32,
                                kind="ExternalInput")
        valv_t = nc.dram_tensor("valv", [128, tot], f32,
                                kind="ExternalInput")
    out_t = nc.dram_tensor("out", [PAD_ROWS, D], f32, kind="ExternalOutput")

    max_nch = 1
    max_sup = 1
    for s0 in range(0, N_BLOCKS, SUPER):
        blocks = range(s0, min(s0 + SUPER, N_BLOCKS))
        sup_tot = 0
        for r in range(N_RANGES):
            nch = int(sum(cpbr[b, r] for b in blocks))
            max_nch = max(max_nch, nch)
            sup_tot += nch
        max_sup = max(max_sup, sup_tot)

    with tile.TileContext(nc) as tc:
        with (
            tc.tile_pool(name="const", bufs=1) as cpool,
            tc.tile_pool(name="meta", bufs=1) as mpool,
            tc.tile_pool(name="idxp", bufs=max(2, xg_bufs - 1)) as idxpool,
            tc.tile_pool(name="xg", bufs=xg_bufs) as xgpool,
            tc.tile_pool(name="sv", bufs=sv_bufs) as svpool,
            tc.tile_pool(name="agg", bufs=3) as aggpool,
            tc.tile_pool(name="osb", bufs=3) as opool,
            tc.tile_pool(name="psA", bufs=SUPER, space="PSUM") as psa,
            tc.tile_pool(name="psB", bufs=2, space="PSUM") as psb,
        ):
            nc.gpsimd.load_library(library_config.mlp)
            w_sb = cpool.tile([128, 128], f32, tag="w")
            nc.sync.dma_start(out=w_sb[:], in_=w_t.ap())
            idx_sb = None
            if not idx_stream:
                idx_sb = mpool.tile([128, tot * 8], mybir.dt.int16, tag="idx")
                nc.sync.dma_start(out=idx_sb[:], in_=idx_t.ap())
            if sv_mode != "host":
                iota_sb = mpool.tile([128, 128], bf16, tag="iota")
                nc.sync.dma_start(out=iota_sb[:], in_=iota_t.ap())
                dstv_sb = mpool.tile([128, tot], f32, tag="dstv")
                nc.sync.dma_start(out=dstv_sb[:], in_=dstv_t.ap())
                valv_sb = mpool.tile([128, tot], f32, tag="valv")
                nc.sync.dma_start(out=valv_sb[:], in_=valv_t.ap())

            xg_const = None
            if ab_gather:
                xg_const = cpool.tile([128, max_nch, 128], bf16, tag="xgc")
                for j in range(max_nch):
                    nc.sync.dma_start(out=xg_const[:, j, :],
                                      in_=x_t.ap()[0:128, :])
            sv_const = None
            if ab_sv:
                sv_const = cpool.tile([128, max_nch, 128], bf16, tag="svc")
                for j in range(max_nch):
                    nc.sync.dma_start(
                        out=sv_const[:, j, :],
                        in_=x_t.ap()[0:128, :])

            def body():
              for s0 in range(0, N_BLOCKS, SUPER):
                blocks = list(range(s0, min(s0 + SUPER, N_BLOCKS)))
                sup_first = int(chunk_off[blocks[0], 0])
                sup_nch = int(sum(cpbr[b, r] for b in blocks
                                  for r in range(N_RANGES)))
                sv_sup = None
                if sv_super and not ab_sv and sv_mode == "host":
                    sv_sup = svpool.tile([128, max_sup, 128], bf16, tag="svs")
                    nc.sync.dma_start(
                        out=sv_sup[:, :sup_nch, :],
                        in_=svf_t.ap()[:, sup_first * 128
                                       : (sup_first + sup_nch) * 128],
                    )
                osb_sup = None
                if out_batch and not (ab_tail or ab_mm):
                    osb_sup = opool.tile([128, SUPER, 128], f32, tag="osbs")
                # one gather + one bulk sv load per source range covering all
                # blocks of this super-block (their chunks are contiguous)
                gathers = {}  # r -> (xg_tile, sv_tile, first_chunk)
                for r in range(N_RANGES):
                    nch = int(sum(cpbr[b, r] for b in blocks))
                    if nch == 0:
                        continue
                    first = int(chunk_off[blocks[0], r])
                    if ab_gather:
                        xg = xg_const
                    else:
                        ew = 2 if ab_wide else 1  # rows per gathered elem
                        xg = xgpool.tile([128, max_nch, 128 * ew], bf16,
                                         tag="xg")
                        base = r * RANGE
                        rows = min(RANGE, N_NODES - base)
                        in_ap = (x2_t.ap() if ab_wide
                                 else x_t.ap()[base : base + rows, :])
                        if idx_stream:
                            idxw = idxpool.tile([128, max_nch * 8],
                                                mybir.dt.int16, tag="idxw")
                            nc.sync.dma_start(
                                out=idxw[:, : nch * 8],
                                in_=idx_t.ap()[:, first * 8 : (first + nch) * 8],
                            )
                        pieces = min(split, nch)
                        c0 = 0
                        for p in range(pieces):
                            n_p = nch // pieces + (1 if p < nch % pieces else 0)
                            nc.gpsimd.dma_gather(
                                out_ap=xg[:, c0 : c0 + n_p, :],
                                in_ap=in_ap,
                                idxs_ap=(
                                    idxw[:, c0 * 8 : (c0 + n_p) * 8]
                                    if idx_stream else
                                    idx_sb[
                                        :, (first + c0) * 8 : (first + c0 + n_p) * 8]),
                                num_idxs=n_p * 128,
                                num_idxs_reg=n_p * 128,
                                elem_size=D * ew,
                                single_packet=single_packet,
                                queue_num=qassign[(s0, r, p)],
                            )
                            c0 += n_p
                    if ab_sv:
                        sv = sv_const
                    elif sv_mode != "host":
                        sv = None
                    elif sv_super:
                        sv = None  # use sv_sup
                    else:
                        sv = svpool.tile([128, max_nch, 128], bf16, tag="sv")
                        nc.sync.dma_start(
                            out=sv[:, :nch, :],
                            in_=svf_t.ap()[:, first * 128 : (first + nch) * 128],
                        )
                    gathers[r] = (xg, sv, first)

                psum = {b: psa.tile([128, 128], f32, tag="aggps",
                                    name=f"aggps{b}")
                        for b in blocks}
                done = {b: 0 for b in blocks}
                for r in range(N_RANGES):
                    if r not in gathers:
                        continue
                    xg, sv, first = gathers[r]
                    for b in blocks:
                        for k in range(int(cpbr[b, r])):
                            j = int(chunk_off[b, r]) + k
                            col = j - first
                            if ab_sv:
                                rhs = sv[:, col, :]
                            elif sv_mode == "host":
                                if sv_super:
                                    rhs = sv_sup[:, j - sup_first, :]
                                else:
                                    rhs = sv[:, col, :]
                            else:
                                svc = svpool.tile([128, 128], bf16, tag="svd")
                                nc.vector.tensor_scalar(
                                    out=svc[:],
                                    in0=iota_sb[:],
                                    scalar1=dstv_sb[:, j : j + 1],
                                    scalar2=valv_sb[:, j : j + 1],
                                    op0=mybir.AluOpType.is_equal,
                                    op1=mybir.AluOpType.mult,
                                )
                                rhs = svc[:]
                            if not ab_mm:
                                nc.tensor.matmul(
                                    out=psum[b][:],
                                    lhsT=xg[:, col, :],
                                    rhs=rhs,
                                    start=(done[b] == 0),
                                    stop=(done[b] == int(nch_block[b]) - 1),
                                )
                            done[b] += 1
                            if done[b] == int(nch_block[b]) and not (
                                    ab_tail or ab_mm):
                                agg_sb = aggpool.tile([128, 128], f32,
                                                      tag="aggsb")
                                nc.scalar.activation(
                                    out=agg_sb[:], in_=psum[b][:],
                                    func=mybir.ActivationFunctionType.Copy,
                                )
                                out_ps = psb.tile([128, 128], f32, tag="outps")
                                nc.tensor.matmul(
                                    out=out_ps[:], lhsT=agg_sb[:], rhs=w_sb[:],
                                    start=True, stop=True,
                                )
                                if out_batch:
                                    nc.scalar.activation(
                                        out=osb_sup[:, b - s0, :],
                                        in_=out_ps[:],
                                        func=mybir.ActivationFunctionType.Relu,
                                    )
                                else:
                                    out_sb = opool.tile([128, 128], f32,
                                                        tag="osb")
                                    nc.scalar.activation(
                                        out=out_sb[:], in_=out_ps[:],
                                        func=mybir.ActivationFunctionType.Relu,
                                    )
                                    nc.sync.dma_start(
                                        out=out_t.ap()[
                                            b * 128 : (b + 1) * 128, :],
                                        in_=out_sb[:],
                                    )
                if out_batch and not (ab_tail or ab_mm):
                    nb = len(blocks)
                    out_view = out_t.ap()[
                        s0 * 128 : (s0 + nb) * 128, :
                    ].rearrange("(j p) f -> p j f", p=128)
                    nc.sync.dma_start(
                        out=out_view,
                        in_=osb_sup[:, :nb, :],
                    )

            if hw_loop and repeat > 1:
                he = ((mybir.EngineType.PE, mybir.EngineType.Pool,
                       mybir.EngineType.Activation, mybir.EngineType.SP)
                      if hints else ())
                with tc.For_i(0, repeat, 1, hint_engines=he):
                    body()
            else:
                for _rep in range(repeat):
                    body()

    nc.compile()
    return nc


def _prep_inputs(input_features, weight, edge_vals, edge_src, edge_dst):
    cpbr, tot, idx16, svf, dst_T, val_T = _schedule(
        np.asarray(edge_src), np.asarray(edge_vals), np.asarray(edge_dst)
    )
    x_bf = np.asarray(input_features).astype(ml_dtypes.bfloat16)
    w = np.ascontiguousarray(np.asarray(weight, dtype=np.float32))
    iota = np.tile(np.arange(128, dtype=np.float32), (128, 1)).astype(
        ml_dtypes.bfloat16)
    in_maps = []
    for c in range(N_CORES):
        in_maps.append({
            "xbf": x_bf,
            "w": w,
            "iota": iota,
            "idx": np.ascontiguousarray(idx16[c]),
            "svf": np.ascontiguousarray(svf[c]),
            "dstv": np.ascontiguousarray(dst_T[c]),
            "valv": np.ascontiguousarray(val_T[c]),
        })
    return cpbr, tot, in_maps


# best-known program configuration (updated as benchmarks come in)
BEST = dict(balance=True, split=2, sv_super=True, sv_bufs=2, out_batch=True,
            hints=True)


def kernel(input_features, weight, edge_vals, edge_src, edge_dst):
    cpbr, tot, in_maps = _prep_inputs(
        input_features, weight, edge_vals, edge_src, edge_dst
    )
    nc = _build_program(tuple(int(x) for x in cpbr.reshape(-1)), **BEST)
    res = run_bass_kernel_spmd(nc, in_maps, list(range(N_CORES)))
    out = np.concatenate(
        [res.results[c]["out"][:ROWS_PER_CORE] for c in range(N_CORES)], axis=0
    )
    return out.astype(np.float32)
